# revision 13
# baseline (speedup 1.0000x reference)
"""Cross-attention block kernel for Trainium2 (8 NeuronCores, data-parallel over batch).

Reference computation (per batch element b):
    Q = q[b] @ Wq; K = k[b] @ Wk; V = v[b] @ Wv        # [4096, 128] each
    O = softmax(Q @ K^T / sqrt(128)) @ V               # [4096, 128]

Sharding: one batch element per core (B == n_cores == 8), weights replicated.

v2 design (vs the fp32r/fp16 v1 at 219us):
  - Inputs are cast to bf16 on the host and shipped feature-major
    (qT/kT/vT = [1024, 4096] bf16): input DMA halves to ~75us.
  - Projections in bf16 (PSUM fp32, DVE-copied to bf16 SBUF): QT/KT tiles
    [AD=128, 512] per chunk.
  - Scores TRANSPOSED in bf16: ST[kv,q-chunk] = KT-tile.T @ QT-chunk, two
    kv tiles (one "pair") per PSUM tile [128, 2, 512].
  - E = exp(ST/sqrt(128) - 3) on ACT, written directly as fp8e4 into
    per-pair SBUF tiles [128, 2, 512] (the -3 bias keeps exp < 240 = fp8e4
    max for |score| up to 8.5 sigma; it cancels in the softmax division).
  - PV in fp8e4 DoubleRow (2x PE throughput): stationary = E pair
    [Ki=128, 2, q=128], moving = V_aug pair [Ki=128, 2, 129] where column
    128 is ones -> out[q=128, 129] accumulates both the unnormalized
    output AND the softmax denominator, q-major (no output transposes).
    V_aug is built fp8 during the v projections via PE transposes.
  - Normalize on DVE (reciprocal of col 128, tensor_scalar_mul), DMA out.

Cost model budget per core: PE ~113us (proj 41 + scores 55 + PV 14 + misc),
ACT 8*16 activations of 1024+222 cycles ~ 134us (the bound), DMA ~77us,
DVE ~50us. Greedy interleaver keeps ACT gapless after a ~8us DMA ramp.
"""

import os
import sys

import numpy as np

for _p in ("/opt/trn_rl_repo",):
    if _p not in sys.path and os.path.isdir(_p):
        sys.path.insert(0, _p)

import concourse.bacc as bacc
import concourse.tile as tile
from concourse import mybir
from concourse.bass_utils import run_bass_kernel_spmd
from concourse.masks import make_identity

F32 = mybir.dt.float32
F32R = mybir.dt.float32r
BF16 = mybir.dt.bfloat16
FP16 = mybir.dt.float16

B, NQ, NKV, QD, KVD, AD = 8, 4096, 4096, 1024, 1024, 128
P = 128
FT = QD // P          # 8 feature tiles (projection contraction tiles)
QCHUNK = 512
NQC = NQ // QCHUNK    # 8 chunks (q side and kv side)
TPC = QCHUNK // P     # 4 tiles per chunk
NPAIR = NKV // (2 * P)  # 16 kv pairs
VW = 144              # V_aug pair-half width (128 vals + ones col + pad to 16B)
SCALE = float(AD) ** -0.5
EBIAS = -3.0          # exp(s - 3): keeps exp under fp8e4 max (240) to 8.5 sigma
E_SLOTS = 36

TRACE = False         # test.py flips this for profiling runs
_TRACE_KW = {}


def build_bass():
    nc = bacc.Bacc("TRN2", target_bir_lowering=False, debug=False, num_devices=B)

    qT = nc.dram_tensor("qT", [QD, NQ], F32, kind="ExternalInput")
    kT = nc.dram_tensor("kT", [KVD, NKV], F32, kind="ExternalInput")
    vT = nc.dram_tensor("vT", [KVD, NKV], BF16, kind="ExternalInput")
    wq = nc.dram_tensor("wq", [QD, AD], F32, kind="ExternalInput")
    wk = nc.dram_tensor("wk", [KVD, AD], F32, kind="ExternalInput")
    wv = nc.dram_tensor("wv", [KVD, AD], BF16, kind="ExternalInput")
    out = nc.dram_tensor("out", [NQ, AD], F32, kind="ExternalOutput")

    DR = mybir.MatmulPerfMode.DoubleRow

    with tile.TileContext(nc) as tc:
        with (
            tc.tile_pool(name="const", bufs=1) as const,
            tc.tile_pool(name="persist", bufs=1) as persist,
            tc.tile_pool(name="xin", bufs=3) as xin,
            tc.tile_pool(name="work", bufs=2) as work,
            tc.tile_pool(name="epool", bufs=E_SLOTS) as epool,
            tc.tile_pool(name="ps_sc", bufs=2, space="PSUM") as ps_sc,
            tc.tile_pool(name="ps_pv", bufs=2, space="PSUM") as ps_pv,
            tc.tile_pool(name="ps_pj", bufs=2, space="PSUM") as ps_pj,
        ):
            # ---- weights / identity
            w_sb = {}
            for name, w, wdt in (("q", wq, F32R), ("k", wk, F32R),
                                 ("v", wv, BF16)):
                t = const.tile([P, FT, AD], wdt, tag=f"w{name}", name=f"w{name}")
                ap = w.ap().rearrange("(t p) a -> p t a", p=P)
                if wdt is F32R:
                    ap = ap.bitcast(F32R)
                nc.sync.dma_start(out=t, in_=ap)
                w_sb[name] = t
            ident = const.tile([P, P], F32)
            make_identity(nc, ident)
            ebias = const.tile([P, 1], F32, tag="ebias", name="ebias")
            nc.vector.memset(ebias, EBIAS)

            # ---- per-chunk persistent tiles (separate tags => chunk-granular deps)
            qt_t = [persist.tile([P, QCHUNK], F32R, tag=f"qt{c}", name=f"qt{c}")
                    for c in range(NQC)]
            kt_t = [persist.tile([P, QCHUNK], F32R, tag=f"kt{c}", name=f"kt{c}")
                    for c in range(NQC)]
            # V_aug, fp8, packed for DoubleRow: [part, local pair, half, col]
            vpk_t = [persist.tile([P, 2, 2, VW], FP16, tag=f"vn{c}", name=f"vn{c}")
                     for c in range(NQC)]

            def proj_chunk(src_dram, which, c):
                """DMA + project one 512-wide chunk c; returns psum ap [AD, 512]."""
                xdt = BF16 if which == "v" else F32R
                src = xin.tile([P, FT, QCHUNK], xdt, tag="xin", name="xin")
                ap = (src_dram.ap()[:, c * QCHUNK:(c + 1) * QCHUNK]
                      .rearrange("(t p) n -> p t n", p=P))
                if xdt is F32R:
                    ap = ap.bitcast(F32R)
                nc.sync.dma_start(out=src, in_=ap)
                pp = ps_pj.tile([P, QCHUNK], F32, tag="pj", name="pp")
                for t in range(FT):
                    nc.tensor.matmul(
                        pp, w_sb[which][:, t, :], src[:, t, :],
                        start=(t == 0), stop=(t == FT - 1),
                    )
                return pp

            def k_chunk(c):
                nc.vector.tensor_copy(kt_t[c], proj_chunk(kT, "k", c))

            def q_chunk(c):
                nc.vector.tensor_copy(qt_t[c], proj_chunk(qT, "q", c))

            def v_chunk(c):
                pp = proj_chunk(vT, "v", c)
                vt = work.tile([P, QCHUNK], F32, tag="vt", name="vt")
                nc.vector.tensor_copy(vt, pp)
                nc.vector.memset(vpk_t[c][:, :, :, AD:AD + 1], 1.0)
                tp = ps_pj.tile([P, QCHUNK], F32, tag="pj", name="tp")
                for j in range(TPC):
                    nc.tensor.transpose(
                        tp[:, j * P:(j + 1) * P], vt[:, j * P:(j + 1) * P], ident)
                for j in range(TPC):
                    nc.vector.tensor_copy(
                        vpk_t[c][:, j // 2, j % 2, 0:AD],
                        tp[:, j * P:(j + 1) * P])

            # ---- spine: DMA + projection units, hand-ordered for startup
            spine = [
                ("k", 0), ("q", 0), ("k", 1), ("q", 1), ("k", 2), ("q", 2),
                ("k", 3), ("k", 4), ("v", 0), ("k", 5), ("v", 1), ("k", 6),
                ("v", 2), ("k", 7), ("v", 3), ("v", 4), ("q", 3), ("v", 5),
                ("q", 4), ("v", 6), ("q", 5), ("v", 7), ("q", 6), ("q", 7),
            ]
            spine_pos = 0
            kt_done = [False] * NQC
            qt_done = [False] * NQC
            vn_done = [False] * NQC

            def emit_spine():
                nonlocal spine_pos
                kind, c = spine[spine_pos]
                spine_pos += 1
                if kind == "k":
                    k_chunk(c)
                    kt_done[c] = True
                elif kind == "q":
                    q_chunk(c)
                    qt_done[c] = True
                else:
                    v_chunk(c)
                    vn_done[c] = True

            def emit_spine_v(vc):
                """Force-emit spine units up to and including ('v', vc)."""
                while spine_pos < len(spine):
                    unit = spine[spine_pos]
                    emit_spine()
                    if unit == ("v", vc):
                        return True
                return False

            # ---- E tiles (per chunk x kv-pair), exp scores in fp8
            E_tiles = {}

            def emit_st(c, p):
                sp = ps_sc.tile([P, 2, QCHUNK], F32, tag="sc", name="sp")
                for h in range(2):
                    kv = 2 * p + h
                    nc.tensor.matmul(
                        sp[:, h, :],
                        kt_t[kv // TPC][:, (kv % TPC) * P:(kv % TPC + 1) * P],
                        qt_t[c],
                        start=True, stop=True,
                    )
                E = epool.tile([P, 2, QCHUNK], FP16, tag="E", name=f"E{c}_{p}")
                nc.scalar.activation(
                    out=E, in_=sp,
                    func=mybir.ActivationFunctionType.Exp,
                    scale=SCALE, bias=ebias,
                )
                E_tiles[(c, p)] = E

            # ---- PV stream: per chunk, 4 SEQUENTIAL q-tile passes over the
            # 16 kv pairs (PSUM allows only one open accumulation group per
            # 2KB bank/zero-region; passes ping-pong the two ps_pv banks).
            pl_tiles = {}

            def emit_pv(c, j, p):
                if p == 0:
                    pl_tiles[(c, j)] = ps_pv.tile(
                        [P, 2 * AD], F32, tag="pv", name=f"pl{c}_{j}")
                pl = pl_tiles[(c, j)]
                E = E_tiles[(c, p)]
                for h in range(2):
                    nc.tensor.matmul(
                        pl[:, 0:AD + 1],
                        E[:, h, j * P:(j + 1) * P],
                        vpk_t[p // 2][:, p % 2, h, 0:AD + 1],
                        start=(p == 0 and h == 0),
                        stop=(p == NPAIR - 1 and h == 1),
                    )
                if j == TPC - 1:
                    E_tiles.pop((c, p))
                if p == NPAIR - 1:
                    recip = work.tile([P, 1], F32, tag="recip", name="recip")
                    nc.vector.reciprocal(recip, pl[:, AD:AD + 1])
                    o_sb = work.tile([P, AD], F32, tag="o", name="o")
                    nc.vector.tensor_scalar_mul(o_sb, pl[:, 0:AD], recip)
                    r0 = c * QCHUNK + j * P
                    nc.sync.dma_start(out=out.ap()[r0:r0 + P, :], in_=o_sb)
                    del pl_tiles[(c, j)]

            # ---- greedy interleaver ------------------------------------------
            # scores stream: chunks 0-1 ride the k-DMA order (pair-major),
            # rest chunk-major.
            st_units = (
                [(c, p) for p in range(NPAIR) for c in range(2)]
                + [(c, p) for c in range(2, NQC) for p in range(NPAIR)]
            )
            st_pos = 0
            # PV chains run CHUNK-SERIAL (one open accumulation chain at a
            # time): ps_pv bufs=2 holds exactly one chunk's two tiles, so a
            # second open chain would deadlock the in-order PE queue. The E
            # pool recycles buffers by allocation index, so before E
            # allocation N the PV reader of allocation N-E_SLOTS must already
            # be emitted on the PE queue (exact victim gating below).
            e_order = []                 # E allocations in order
            consumed = set()
            pv_state = [0, 0, 0]         # [active chunk, q-tile pass, pair]

            def emit_pv_next(force):
                c, j, p = pv_state
                if c >= NQC or (c, p) not in E_tiles:
                    return False
                if not vn_done[p // 2]:
                    if not force:
                        return False
                    if not emit_spine_v(p // 2):
                        raise RuntimeError("v spine exhausted")
                emit_pv(c, j, p)
                if j == TPC - 1:
                    consumed.add((c, p))
                if p < NPAIR - 1:
                    pv_state[2] = p + 1
                elif j < TPC - 1:
                    pv_state[1] = j + 1
                    pv_state[2] = 0
                else:
                    pv_state[0] += 1
                    pv_state[1] = 0
                    pv_state[2] = 0
                return True

            def drain_pv():
                while emit_pv_next(force=False):
                    pass

            while st_pos < len(st_units) or pv_state[0] < NQC:
                # spine pacing: <=2 units between score ticks, early bias
                budget = 2
                while spine_pos < len(spine) and budget > 0 \
                        and spine_pos * 5 <= st_pos * 4 + 24:
                    emit_spine()
                    budget -= 1

                drain_pv()

                if st_pos < len(st_units):
                    c, p = st_units[st_pos]
                    n = len(e_order)
                    if n >= E_SLOTS:
                        victim = e_order[n - E_SLOTS]
                        while victim not in consumed:
                            if not emit_pv_next(force=True):
                                raise RuntimeError(
                                    f"cannot free E victim {victim} "
                                    f"pv={pv_state} st={st_pos}")
                    if kt_done[p // 2] and qt_done[c]:
                        emit_st(c, p)
                        e_order.append((c, p))
                        st_pos += 1
                    else:
                        # scores blocked on projections: advance the spine
                        if spine_pos < len(spine):
                            emit_spine()
                        else:
                            raise RuntimeError("scores blocked after spine end")
                elif not emit_pv_next(force=True) and pv_state[0] < NQC:
                    raise RuntimeError(f"pv wedged at tail: {pv_state}")

            assert not E_tiles, E_tiles.keys()
            assert not pl_tiles, pl_tiles.keys()

    nc.compile()
    return nc


_NC_CACHE = None


def _to_bf16(a):
    import ml_dtypes
    return np.ascontiguousarray(a.astype(ml_dtypes.bfloat16))


def kernel(q, k, v, Wq, Wk, Wv):
    global _NC_CACHE
    q = np.asarray(q, dtype=np.float32)
    k = np.asarray(k, dtype=np.float32)
    v = np.asarray(v, dtype=np.float32)
    Wq = np.asarray(Wq, dtype=np.float32)
    Wk = np.asarray(Wk, dtype=np.float32)
    Wv = np.asarray(Wv, dtype=np.float32)

    wq_c = np.ascontiguousarray(Wq)
    wk_c = np.ascontiguousarray(Wk)
    wv_b = _to_bf16(Wv)
    # Shard: batch b -> core b; feature-major layout (q/k fp32, v bf16).
    in_maps = []
    for b in range(B):
        in_maps.append({
            "qT": np.ascontiguousarray(q[b].T),
            "kT": np.ascontiguousarray(k[b].T),
            "vT": _to_bf16(v[b].T),
            "wq": wq_c, "wk": wk_c, "wv": wv_b,
        })

    if _NC_CACHE is None:
        _NC_CACHE = build_bass()
    nc = _NC_CACHE

    res = None
    for attempt in range(3):
        try:
            res = run_bass_kernel_spmd(
                nc, in_maps, core_ids=list(range(B)), trace=TRACE, **_TRACE_KW
            )
            break
        except Exception:
            # rare transient NRT_EXEC_UNIT_UNRECOVERABLE on dispatch; retry
            if attempt == 2:
                raise
            import time as _time
            _time.sleep(5)
    if TRACE:
        kernel.last_results = res

    out = np.stack([res.results[b]["out"] for b in range(B)], axis=0)
    return out
